# revision 2
# baseline (speedup 1.0000x reference)
"""GATEncoder kernel for 8 Trainium2 NeuronCores.

Strategy (hardcoded for the nn_GATEncoder problem):
  - Only nodes < batch_size (8192) reach the output, so aggregation/decoder
    run for 8192 target nodes, sharded 1024 per core (8 windows of 128).
  - Encoder + GAT projection (xs, a_src, a_dst) are replicated on every core
    for all 10000 nodes (cheaper than cross-core collectives on this chip).
    Each core builds a node-major DRAM table T[10240, 576] =
    [xs(512) | a_src(2) | a_dst(2) | pad], in a per-core node permutation
    that puts the core's 1024 target nodes first (makes all local slices
    static).
  - Edges are partitioned by target core, bucketed into 8 windows of 128
    targets, sorted by source within a window, padded to a uniform
    per-window tile count (same static schedule on every core).
  - Per 128-edge tile: dma_gather pulls xs[src]/a_src[src] rows from T;
    attention logits e = leaky(a_src[src] + a_dst[tgt]); the a_dst[tgt]
    broadcast and the segment softmax/scatter-add are done with one-hot
    matmuls into PSUM (exact for duplicate edges).
  - Epilogue (skip, ELU, decoder) feature-major on the local 1024 nodes.
"""

import math

import numpy as np

N_NODES = 10000
NPAD = 10240
N_EDGES = 160000
N_IN, H, HEADS, HOUT = 128, 256, 2, 256
BATCH = 8192
NCORES = 8
TPC = BATCH // NCORES          # 1024 target nodes per core
P = 128
WPC = TPC // P                 # 8 windows per core
ROW = 576                      # table row: 512 xs + 2 a_src + 2 a_dst + 60 pad
F32 = np.float32

_cache = {}


# ----------------------------------------------------------------------------
# Host-side preprocessing: edge partitioning / permutation / schedules
# ----------------------------------------------------------------------------

def _prepare_edges(edge_index):
    src = np.asarray(edge_index[0], dtype=np.int64)
    tgt = np.asarray(edge_index[1], dtype=np.int64)
    loops = np.arange(N_NODES, dtype=np.int64)
    src = np.concatenate([src, loops])
    tgt = np.concatenate([tgt, loops])
    keep = tgt < BATCH
    src, tgt = src[keep], tgt[keep]

    core = tgt // TPC
    tloc = tgt - core * TPC
    win = tloc // P
    trel = tloc - win * P

    # per (core, window) edge lists sorted by source
    buckets = {}
    counts = np.zeros((NCORES, WPC), dtype=np.int64)
    for c in range(NCORES):
        m = core == c
        sc, wc, rc = src[m], win[m], trel[m]
        for w in range(WPC):
            mw = wc == w
            s, r = sc[mw], rc[mw]
            o = np.argsort(s, kind="stable")
            buckets[(c, w)] = (s[o], r[o])
            counts[c, w] = s.size

    tiles_per_win = [int(math.ceil(counts[:, w].max() / P)) for w in range(WPC)]
    tiles_per_win = [max(t, 1) for t in tiles_per_win]
    return buckets, tiles_per_win


def _per_core_arrays(buckets, tiles_per_win, c):
    """Returns (gather_idx int16 wrapped, tgt_rel f32 [P, TILES],
    onehotT u8 [TILES, P, P], perm order)."""
    ntiles = sum(tiles_per_win)
    srcs = np.zeros(ntiles * P, dtype=np.int64)      # padded slots gather row 0
    trel = np.full(ntiles * P, -1.0, dtype=F32)      # -1 -> contributes nothing
    t0 = 0
    for w in range(WPC):
        s, r = buckets[(c, w)]
        n = s.size
        base = t0 * P
        srcs[base : base + n] = s
        trel[base : base + n] = r.astype(F32)
        t0 += tiles_per_win[w]

    # node permutation: core's targets first, then all other nodes
    targets = np.arange(c * TPC, (c + 1) * TPC, dtype=np.int64)
    others_mask = np.ones(N_NODES, dtype=bool)
    others_mask[targets] = False
    order = np.concatenate([targets, np.nonzero(others_mask)[0]])
    pos = np.empty(N_NODES, dtype=np.int64)
    pos[order] = np.arange(N_NODES)

    gidx = pos[srcs].astype(np.int16)                # table row per edge slot
    # wrap int16 indices: element i at [i % 16, i // 16], replicated to 128 rows
    tot = gidx.size
    wrapped = gidx.reshape(tot // 16, 16).T          # [16, tot/16]
    wrapped = np.tile(wrapped, (8, 1)).copy()        # [128, tot/16]

    trel_mat = trel.reshape(ntiles, P).T.copy()      # [P, TILES]

    # transposed one-hot per tile: [t, j, p] = (trel[t, p] == j)
    tr = trel.reshape(ntiles, P)
    iota = np.arange(P, dtype=F32)
    ohT = (tr[:, None, :] == iota[None, :, None]).astype(np.uint8)  # [T, j, p]
    return wrapped, trel_mat, np.ascontiguousarray(ohT), order


# ----------------------------------------------------------------------------
# Bass program
# ----------------------------------------------------------------------------

def _split_multi_waits(nc):
    """This walrus build encodes at most one sync wait per instruction; hoist
    extra waits into standalone EventSemaphore ops on the same engine."""
    import concourse.mybir as mybir

    n = 0
    for f in nc.m.functions:
        for blk in f.blocks:
            out = []
            for ins in list(blk.instructions):
                si = ins.sync_info
                if si is not None and si.on_wait is not None \
                        and len(si.on_wait) > 1:
                    for w in list(si.on_wait[:-1]):
                        ev = mybir.InstEventSemaphore(
                            name=f"{ins.name}_w{n}", ins=[], outs=[])
                        ev.engine = ins.engine
                        ev.sync_info = mybir.SyncInfo(on_wait=[w],
                                                      on_update=[])
                        out.append(ev)
                        n += 1
                    si.on_wait = [si.on_wait[-1]]
                out.append(ins)
            blk.instructions = out


def _build_program(tiles_per_win):
    import concourse.bacc as bacc
    import concourse.mybir as mybir
    import concourse.tile as tile

    dt = mybir.dt
    Alu = mybir.AluOpType
    Act = mybir.ActivationFunctionType

    TILES = sum(tiles_per_win)
    NB = 20                     # node blocks in phase A
    BN = NPAD // NB             # 512 nodes per block
    NT = BN // P                # 4 node chunks of 128 per block

    GT = 8                      # tiles per gather call / onehot group
    NGRP = math.ceil(TILES / GT)

    nc = bacc.Bacc("TRN2", target_bir_lowering=False)

    def inp(name, shape, dtype=dt.float32):
        return nc.dram_tensor(name, shape, dtype, kind="ExternalInput")

    xT = inp("xT", [P, NPAD])
    w1T = inp("w1T", [N_IN, H])
    ln_g = inp("ln_g", [H, 1])
    ln_b = inp("ln_b", [H, 1])
    b1 = inp("b1", [H, 1])
    b2 = inp("b2", [H, 1])
    w2T = inp("w2T", [H, H])
    gatT = inp("gatT", [H, HEADS * HOUT + 4])   # gat_w.T with att cols appended
    skipT = inp("skipT", [H, HEADS * HOUT])
    gat_bias = inp("gat_bias", [HEADS * HOUT, 1])
    skip_b = inp("skip_b", [HEADS * HOUT, 1])
    d1T = inp("d1T", [HEADS * HOUT, 4 * H])
    db1 = inp("db1", [4 * H, 1])
    d2T = inp("d2T", [4 * H, 1])
    db2 = inp("db2", [1, 1])
    gidx = inp("gidx", [P, (TILES * P) // 16], dt.int16)
    meta = inp("meta", [P, TILES])
    ohT_in = inp("ohT", [TILES, P, P], dt.uint8)
    iota_in = inp("iota", [P, P])              # iota[p, j] = j
    negmean_in = inp("negmean", [P, P])        # all -1/256
    posmean_in = inp("posmean", [P, P])        # all +1/256
    ident_in = inp("ident", [P, P])            # identity
    ones_in = inp("ones", [P, 1])

    y_out = nc.dram_tensor("y", [1, TPC], dt.float32, kind="ExternalOutput")

    MM = HEADS * HOUT          # 512
    FC = MM // P               # 4 feature chunks of the GAT output

    with tile.TileContext(nc) as tc:
        with (
            tc.tile_pool(name="const", bufs=1) as cpool,
            tc.tile_pool(name="persist", bufs=1) as ppool,
            tc.tile_pool(name="dram", bufs=1, space="DRAM") as dpool,
        ):
            # ---- constants / weights to SBUF ----
            def load_const(t, shape, dtype=dt.float32):
                s = cpool.tile(shape, dtype, name=t.name, tag=t.name)
                nc.sync.dma_start(out=s[:], in_=t[:])
                return s

            def load_kc(t, k, cols):
                """[k*128, cols] DRAM -> [128, k, cols] SBUF."""
                s = cpool.tile([P, k, cols], dt.float32, name=t.name,
                               tag=t.name)
                nc.sync.dma_start(
                    out=s[:], in_=t[:].rearrange("(k p) c -> p k c", p=P))
                return s

            iota_m = load_const(iota_in, [P, P])
            negmean = load_const(negmean_in, [P, P])
            posmean = load_const(posmean_in, [P, P])
            ident = load_const(ident_in, [P, P])
            ones_c = load_const(ones_in, [P, 1])
            w1s = load_const(w1T, [N_IN, H])
            w2s = load_kc(w2T, 2, H)
            gats = load_kc(gatT, 2, MM + 4)
            skips = load_kc(skipT, 2, MM)
            d1s = load_kc(d1T, 4, 4 * H)
            d2s = load_kc(d2T, 8, 1)
            lng = load_kc(ln_g, 2, 1)
            lnb = load_kc(ln_b, 2, 1)
            b1s = load_kc(b1, 2, 1)
            b2s = load_kc(b2, 2, 1)
            gbia = load_kc(gat_bias, 4, 1)
            skb = load_kc(skip_b, 4, 1)
            db1s = load_kc(db1, 8, 1)
            db2s = load_const(db2, [1, 1])
            ln01 = cpool.tile([P, 1], dt.float32, name="ln01", tag="ln01")
            nc.gpsimd.memset(ln01[:], float(np.log(0.1)))
            meta_s = load_const(meta, [P, TILES])
            gidx_s = load_const(gidx, [P, (TILES * P) // 16], dt.int16)

            T_tab = dpool.tile([NPAD, ROW], dt.float32, name="T_tab",
                               tag="T_tab")

            # persistent: local h2 (skip input), node-major a_dst, agg result
            h2loc = [ppool.tile([P, TPC], dt.float32, name=f"h2loc{m}",
                                tag=f"h2loc{m}") for m in range(2)]
            adstw = ppool.tile([P, 2 * WPC], dt.float32, name="adstw",
                               tag="adstw")
            aggs = ppool.tile([P, WPC, MM], dt.float32, name="aggs",
                              tag="aggs")

            # ================= Phase A: encoder -> table =================
            with (
                tc.tile_pool(name="wA", bufs=1) as wA,
                tc.tile_pool(name="asmp", bufs=2) as apool,
                tc.tile_pool(name="psA", bufs=2, space="PSUM") as psA,
                tc.tile_pool(name="psA1", bufs=1, space="PSUM") as psA1,
            ):
                for b in range(NB):
                    bsl = slice(b * BN, (b + 1) * BN)
                    xb = wA.tile([P, BN], dt.float32, name="xb", tag="xb",
                                 bufs=2)
                    nc.sync.dma_start(out=xb[:], in_=xT[:, bsl])

                    h1 = wA.tile([P, 2, BN], dt.float32, name="h1", tag="h1", bufs=2)
                    sq = wA.tile([P, 2, BN], dt.float32, name="sq", tag="sq", bufs=2)
                    for m in range(2):
                        ps = psA.tile([P, BN], dt.float32, name="psA",
                                      tag="psA")
                        nc.tensor.matmul(
                            ps[:], lhsT=w1s[:, m * P : (m + 1) * P],
                            rhs=xb[:], start=True, stop=True)
                        nc.scalar.activation(
                            h1[:, m, :], ps[:], Act.Identity,
                            bias=b1s[:, m, 0:1])
                        nc.scalar.activation(sq[:, m, :], h1[:, m, :],
                                             Act.Square)

                    mu = psA1.tile([P, BN], dt.float32, name="muA", tag="muA")
                    ex2 = psA1.tile([P, BN], dt.float32, name="ex2A",
                                    tag="ex2A")
                    for m in range(2):
                        nc.tensor.matmul(mu[:], lhsT=negmean[:],
                                         rhs=h1[:, m, :],
                                         start=(m == 0), stop=(m == 1))
                        nc.tensor.matmul(ex2[:], lhsT=posmean[:],
                                         rhs=sq[:, m, :],
                                         start=(m == 0), stop=(m == 1))
                    # var = (ex2 + eps) - mu^2   (mu holds -mean)
                    musq = wA.tile([P, BN], dt.float32, name="musq",
                                   tag="musq")
                    nc.scalar.activation(musq[:], mu[:], Act.Square)
                    var = wA.tile([P, BN], dt.float32, name="var", tag="var")
                    nc.vector.scalar_tensor_tensor(
                        var[:], ex2[:], 1e-5, musq[:],
                        op0=Alu.add, op1=Alu.subtract)
                    rv = wA.tile([P, BN], dt.float32, name="rv", tag="rv")
                    nc.vector.reciprocal(rv[:], var[:])
                    rstd = wA.tile([P, BN], dt.float32, name="rstd",
                                   tag="rstd")
                    nc.scalar.activation(rstd[:], rv[:], Act.Sqrt)

                    hrelu = wA.tile([P, 2, BN], dt.float32, name="hrelu",
                                    tag="hrelu", bufs=2)
                    for m in range(2):
                        cen = wA.tile([P, BN], dt.float32, name="cen",
                                      tag="cen")
                        nc.vector.tensor_add(cen[:], h1[:, m, :], mu[:])
                        cn = wA.tile([P, BN], dt.float32, name="cn", tag="cn")
                        nc.vector.tensor_mul(cn[:], cen[:], rstd[:])
                        nc.scalar.activation(
                            hrelu[:, m, :], cn[:], Act.Relu,
                            bias=lnb[:, m, 0:1], scale=lng[:, m, 0:1])

                    h2 = wA.tile([P, 2, BN], dt.float32, name="h2", tag="h2", bufs=2)
                    for m in range(2):
                        ps = psA.tile([P, BN], dt.float32, name="psA",
                                      tag="psA")
                        for k in range(2):
                            nc.tensor.matmul(
                                ps[:], lhsT=w2s[:, k, m * P : (m + 1) * P],
                                rhs=hrelu[:, k, :],
                                start=(k == 0), stop=(k == 1))
                        nc.scalar.activation(
                            h2[:, m, :], ps[:], Act.Identity,
                            bias=b2s[:, m, 0:1])

                    if b * BN < TPC:  # blocks covering the local 1024 targets
                        lo = b * BN
                        for m in range(2):
                            nc.vector.tensor_copy(
                                h2loc[m][:, lo : lo + BN], h2[:, m, :])

                    # xs (+ attention scalars) feature-major
                    xs = wA.tile([P, FC, BN], dt.float32, name="xs", tag="xs", bufs=2)
                    av = wA.tile([4, BN], dt.float32, name="av", tag="av")
                    for f in range(FC):
                        ps = psA.tile([P, BN], dt.float32, name="psA",
                                      tag="psA")
                        for k in range(2):
                            nc.tensor.matmul(
                                ps[:], lhsT=gats[:, k, f * P : (f + 1) * P],
                                rhs=h2[:, k, :],
                                start=(k == 0), stop=(k == 1))
                        nc.scalar.copy(xs[:, f, :], ps[:])
                    pa = psA1.tile([4, BN], dt.float32, name="pavA",
                                   tag="pavA")
                    for k in range(2):
                        nc.tensor.matmul(
                            pa[:], lhsT=gats[:, k, MM : MM + 4],
                            rhs=h2[:, k, :], start=(k == 0), stop=(k == 1))
                    nc.vector.tensor_copy(av[:], pa[:])

                    # transpose to node-major rows and store to table
                    asm = apool.tile([P, NT, ROW], dt.float32, name="asm",
                                     tag="asm")
                    nc.gpsimd.memset(asm[:, :, MM + 4 : ROW], 0.0)
                    for t in range(NT):
                        tsl = slice(t * P, (t + 1) * P)
                        for f in range(FC):
                            tp = psA.tile([P, P], dt.float32, name="tpA",
                                          tag="tpA")
                            nc.tensor.transpose(tp[:], xs[:, f, tsl],
                                                ident[:])
                            nc.scalar.copy(
                                asm[:, t, f * P : (f + 1) * P], tp[:])
                        tp4 = psA1.tile([P, 4], dt.float32, name="tp4A",
                                        tag="tp4A")
                        nc.tensor.transpose(tp4[:], av[:, tsl],
                                            ident[:4, :4])
                        nc.vector.tensor_copy(asm[:, t, MM : MM + 4], tp4[:])
                        if b * BN + t * P < TPC:
                            w = (b * BN + t * P) // P
                            nc.vector.tensor_copy(
                                adstw[:, 2 * w : 2 * w + 2], tp4[:, 2:4])
                    dst = T_tab[:].rearrange("(bb tt pp) r -> bb pp tt r",
                                             bb=NB, pp=P)[b]
                    nc.sync.dma_start(out=dst, in_=asm[:])

            # ================= Phase B: edge aggregation =================
            # Per window: two half-window gathers interleaved with their
            # consumers (slot rotation stays acyclic); agg/z accumulate in
            # PSUM across the whole window.
            win_t0 = []
            t0 = 0
            for w in range(WPC):
                win_t0.append(t0)
                t0 += tiles_per_win[w]
            GH = 8      # max tiles per gather call (1024 idx = 64 desc/engine)

            def _chunks(base, n):
                k = math.ceil(n / GH)
                sizes = [n // k + (1 if i < n % k else 0) for i in range(k)]
                out, b0 = [], base
                for s in sizes:
                    out.append((b0, s))
                    b0 += s
                return out

            with (
                tc.tile_pool(name="wB", bufs=3) as wB,
                tc.tile_pool(name="gpool", bufs=3) as gpool,
                tc.tile_pool(name="psB", bufs=2, space="PSUM") as psB,
            ):
                osrc = ohT_in[:].rearrange("t j p -> j t p")
                for w in range(WPC):
                    ntw = tiles_per_win[w]
                    halves = _chunks(win_t0[w], ntw)
                    agg = psB.tile([P, MM], dt.float32, name="aggps",
                                   tag="aggps")
                    zps = psB.tile([P, 2], dt.float32, name="zps", tag="zps")
                    done = 0
                    for hb, hn in halves:
                        if hn == 0:
                            continue
                        gb = gpool.tile([P, GH, ROW], dt.float32, name="gb",
                                        tag="gb")
                        nc.gpsimd.dma_gather(
                            out_ap=gb[:, :hn, :],
                            in_ap=T_tab[:],
                            idxs_ap=gidx_s[:, hb * 8 : (hb + hn) * 8],
                            num_idxs=hn * P,
                            num_idxs_reg=hn * P,
                            elem_size=ROW,
                        )
                        ou8 = wB.tile([P, GH, P], dt.uint8, name="ou8",
                                      tag="ou8")
                        nc.sync.dma_start(out=ou8[:, :hn, :],
                                          in_=osrc[:, hb : hb + hn, :])
                        of = wB.tile([P, GH, P], dt.float32, name="ohf",
                                     tag="ohf")
                        nc.vector.tensor_copy(of[:, :hn, :], ou8[:, :hn, :])

                        dps = psB.tile([P, 2 * GH], dt.float32, name="dps",
                                       tag="dps")
                        for i in range(hn):
                            nc.tensor.matmul(
                                dps[:, 2 * i : 2 * i + 2],
                                lhsT=of[:, i, :],
                                rhs=adstw[:, 2 * w : 2 * w + 2],
                                start=(i == 0), stop=(i == hn - 1),
                                skip_group_check=True)
                        # e = a_src[src] + d ; leaky(0.2); exp
                        esb = wB.tile([P, 2 * GH], dt.float32, name="esb",
                                      tag="esb")
                        nc.vector.tensor_add(
                            esb[:, : 2 * hn].rearrange(
                                "p (t two) -> p t two", two=2),
                            gb[:, :hn, MM : MM + 2],
                            dps[:, : 2 * hn].rearrange(
                                "p (t two) -> p t two", two=2))
                        lk = wB.tile([P, 2 * GH], dt.float32, name="lk",
                                     tag="lk")
                        nc.vector.scalar_tensor_tensor(
                            lk[:, : 2 * hn], esb[:, : 2 * hn], 0.2,
                            esb[:, : 2 * hn], op0=Alu.mult, op1=Alu.max)
                        wexp = wB.tile([P, 2 * GH], dt.float32, name="wexp",
                                       tag="wexp")
                        nc.scalar.activation(wexp[:, : 2 * hn],
                                             lk[:, : 2 * hn], Act.Exp)

                        for i in range(hn):
                            t = hb + i
                            for h in range(HEADS):
                                ohw = wB.tile([P, P], dt.float32, name="ohw",
                                              tag="ohw", bufs=4)
                                nc.vector.scalar_tensor_tensor(
                                    ohw[:], iota_m[:], meta_s[:, t : t + 1],
                                    wexp[:, 2 * i + h : 2 * i + h + 1]
                                    .to_broadcast([P, P]),
                                    op0=Alu.is_equal, op1=Alu.mult)
                                nc.tensor.matmul(
                                    agg[:, h * HOUT : (h + 1) * HOUT],
                                    lhsT=ohw[:],
                                    rhs=gb[:, i, h * HOUT : (h + 1) * HOUT],
                                    start=(done == 0 and h == 0),
                                    stop=(done == ntw - 1 and h == 1),
                                    skip_group_check=True)
                                nc.tensor.matmul(
                                    zps[:, h : h + 1], lhsT=ohw[:],
                                    rhs=ones_c[:],
                                    start=(done == 0 and h == 0),
                                    stop=(done == ntw - 1 and h == 1),
                                    skip_group_check=True)
                            done += 1
                    rz = wB.tile([P, 2], dt.float32, name="rz", tag="rz")
                    nc.vector.reciprocal(rz[:], zps[:])
                    nc.vector.tensor_mul(
                        aggs[:, w, :].rearrange("p (h f) -> p h f", h=HEADS),
                        agg[:].rearrange("p (h f) -> p h f", h=HEADS),
                        rz[:, :, None].to_broadcast([P, HEADS, HOUT]))

            # ================= Phase C: epilogue =================
            with (
                tc.tile_pool(name="wC", bufs=1) as wC,
                tc.tile_pool(name="wC2", bufs=2) as wC2,
                tc.tile_pool(name="psC", bufs=2, space="PSUM") as psC,
            ):
                NWC = TPC // 512    # 2 column chunks of 512 nodes
                convT = wC.tile([P, FC, TPC], dt.float32, name="convT",
                                tag="convT")
                for w in range(WPC):
                    for f in range(FC):
                        tp = psC.tile([P, P], dt.float32, name="tpC",
                                      tag="tpC")
                        nc.tensor.transpose(
                            tp[:], aggs[:, w, f * P : (f + 1) * P], ident[:])
                        nc.scalar.activation(
                            convT[:, f, w * P : (w + 1) * P], tp[:],
                            Act.Identity, bias=gbia[:, f, 0:1])

                outT = wC.tile([P, FC, TPC], dt.float32, name="outT",
                               tag="outT")
                for f in range(FC):
                    for n in range(NWC):
                        nsl = slice(n * 512, (n + 1) * 512)
                        sp = psC.tile([P, 512], dt.float32, name="skps",
                                      tag="skps")
                        for k in range(2):
                            nc.tensor.matmul(
                                sp[:], lhsT=skips[:, k, f * P : (f + 1) * P],
                                rhs=h2loc[k][:, nsl],
                                start=(k == 0), stop=(k == 1))
                        t_sb = wC2.tile([P, 512], dt.float32, name="t_sb",
                                        tag="t_sb")
                        nc.vector.scalar_tensor_tensor(
                            t_sb[:], sp[:], skb[:, f, 0:1],
                            convT[:, f, nsl], op0=Alu.add, op1=Alu.add)
                        mn = wC2.tile([P, 512], dt.float32, name="mn",
                                      tag="mn")
                        nc.vector.tensor_scalar_min(mn[:], t_sb[:], 0.0)
                        ez = wC2.tile([P, 512], dt.float32, name="ez",
                                      tag="ez")
                        nc.scalar.activation(ez[:], mn[:], Act.Exp,
                                             bias=ln01[:, 0:1])
                        rl = wC2.tile([P, 512], dt.float32, name="rl",
                                      tag="rl")
                        nc.scalar.activation(rl[:], t_sb[:], Act.Relu)
                        nc.vector.scalar_tensor_tensor(
                            outT[:, f, nsl], ez[:], -0.1, rl[:],
                            op0=Alu.add, op1=Alu.add)

                dsb = wC.tile([P, 8, TPC], dt.float32, name="dsb", tag="dsb")
                for m in range(8):
                    for n in range(NWC):
                        nsl = slice(n * 512, (n + 1) * 512)
                        ps = psC.tile([P, 512], dt.float32, name="decps",
                                      tag="decps")
                        for k in range(FC):
                            nc.tensor.matmul(
                                ps[:], lhsT=d1s[:, k, m * P : (m + 1) * P],
                                rhs=outT[:, k, nsl],
                                start=(k == 0), stop=(k == FC - 1))
                        tmp = wC2.tile([P, 512], dt.float32, name="dtmp",
                                       tag="dtmp")
                        nc.scalar.activation(
                            tmp[:], ps[:], Act.Identity,
                            bias=db1s[:, m, 0:1])
                        nc.vector.scalar_tensor_tensor(
                            dsb[:, m, nsl], tmp[:], 0.1, tmp[:],
                            op0=Alu.mult, op1=Alu.max)

                ysb = wC.tile([1, TPC], dt.float32, name="ysb", tag="ysb")
                for n in range(NWC):
                    nsl = slice(n * 512, (n + 1) * 512)
                    yp = psC.tile([1, 512], dt.float32, name="yps",
                                  tag="yps")
                    for m in range(8):
                        nc.tensor.matmul(
                            yp[:], lhsT=d2s[:, m, 0:1],
                            rhs=dsb[:, m, nsl],
                            start=(m == 0), stop=(m == 7))
                    nc.scalar.activation(ysb[:, nsl], yp[:], Act.Identity,
                                         bias=db2s[0:1, 0:1])
                nc.sync.dma_start(out=y_out[:], in_=ysb[:])

    nc.compile()
    return nc


# ----------------------------------------------------------------------------
# Driver
# ----------------------------------------------------------------------------

def _consts():
    iota = np.tile(np.arange(P, dtype=F32), (P, 1)).copy()
    negmean = np.full((P, P), -1.0 / H, dtype=F32)
    posmean = np.full((P, P), 1.0 / H, dtype=F32)
    ident = np.eye(P, dtype=F32)
    ones = np.ones((P, 1), dtype=F32)
    return iota, negmean, posmean, ident, ones


def _host_in_maps(inputs, buckets, tiles_per_win):
    x = np.asarray(inputs["x"], dtype=F32)
    enc_w1, enc_b1 = inputs["enc_w1"], inputs["enc_b1"]
    ln_g, ln_b = inputs["ln_g"], inputs["ln_b"]
    enc_w2, enc_b2 = inputs["enc_w2"], inputs["enc_b2"]
    gat_w, att_src, att_dst = inputs["gat_w"], inputs["att_src"], inputs["att_dst"]
    gat_bias, skip_w, skip_b = inputs["gat_bias"], inputs["skip_w"], inputs["skip_b"]
    dec_w1, dec_b1 = inputs["dec_w1"], inputs["dec_b1"]
    dec_w2, dec_b2 = inputs["dec_w2"], inputs["dec_b2"]

    # a_src[n,h] = att_src[h] . xs[n,h,:] = (gat_w[h-block].T @ att_src[h]) . h2
    # -> compose the attention vectors into h2-space columns on the host.
    gw = np.asarray(gat_w, F32)
    asr = np.asarray(att_src, F32).reshape(HEADS, HOUT)
    ads = np.asarray(att_dst, F32).reshape(HEADS, HOUT)
    att = np.zeros((H, 4), dtype=F32)
    for h in range(HEADS):
        blk = gw[h * HOUT : (h + 1) * HOUT, :]        # [HOUT, H]
        att[:, h] = blk.T @ asr[h]
        att[:, 2 + h] = blk.T @ ads[h]
    gatT = np.concatenate([np.ascontiguousarray(gw.T), att], axis=1)

    iota, negmean, posmean, ident, ones = _consts()
    col = lambda v: np.ascontiguousarray(np.asarray(v, F32).reshape(-1, 1))
    common = {
        "w1T": np.ascontiguousarray(np.asarray(enc_w1, F32).T),
        "ln_g": col(ln_g), "ln_b": col(ln_b),
        "b1": col(enc_b1), "b2": col(enc_b2),
        "w2T": np.ascontiguousarray(np.asarray(enc_w2, F32).T),
        "gatT": np.ascontiguousarray(gatT),
        "skipT": np.ascontiguousarray(np.asarray(skip_w, F32).T),
        "gat_bias": col(gat_bias), "skip_b": col(skip_b),
        "d1T": np.ascontiguousarray(np.asarray(dec_w1, F32).T),
        "db1": col(dec_b1),
        "d2T": np.ascontiguousarray(np.asarray(dec_w2, F32).T),
        "db2": col(dec_b2),
        "iota": iota, "negmean": negmean, "posmean": posmean,
        "ident": ident, "ones": ones,
    }

    in_maps = []
    for c in range(NCORES):
        wrapped, trel_mat, ohT, order = _per_core_arrays(
            buckets, tiles_per_win, c)
        m = dict(common)
        xp = np.zeros((NPAD, N_IN), dtype=F32)
        xp[:N_NODES] = x[order]
        m["xT"] = np.ascontiguousarray(xp.T)
        m["gidx"] = wrapped
        m["meta"] = trel_mat
        m["ohT"] = ohT
        in_maps.append(m)
    return in_maps


def prepared(inputs):
    edge_index = np.asarray(inputs["edge_index"])
    buckets, tiles_per_win = _prepare_edges(edge_index)
    key = tuple(tiles_per_win)
    if key not in _cache:
        _cache[key] = _build_program(tiles_per_win)
    nc = _cache[key]
    in_maps = _host_in_maps(inputs, buckets, tiles_per_win)
    return nc, in_maps


def kernel(x, edge_index, batch_size, enc_w1, enc_b1, ln_g, ln_b, enc_w2,
           enc_b2, gat_w, att_src, att_dst, gat_bias, skip_w, skip_b,
           dec_w1, dec_b1, dec_w2, dec_b2, _trace=False):
    edge_index = np.asarray(edge_index)
    buckets, tiles_per_win = _prepare_edges(edge_index)
    key = tuple(tiles_per_win)
    if key not in _cache:
        _cache[key] = _build_program(tiles_per_win)
    nc = _cache[key]

    inputs = dict(x=x, enc_w1=enc_w1, enc_b1=enc_b1, ln_g=ln_g, ln_b=ln_b,
                  enc_w2=enc_w2, enc_b2=enc_b2, gat_w=gat_w, att_src=att_src,
                  att_dst=att_dst, gat_bias=gat_bias, skip_w=skip_w,
                  skip_b=skip_b, dec_w1=dec_w1, dec_b1=dec_b1, dec_w2=dec_w2,
                  dec_b2=dec_b2)
    in_maps = _host_in_maps(inputs, buckets, tiles_per_win)

    from concourse.bass_utils import run_bass_kernel_spmd
    res = run_bass_kernel_spmd(
        nc, in_maps, core_ids=list(range(NCORES)), trace=_trace)

    y = np.concatenate([res.results[c]["y"][0] for c in range(NCORES)])
    out = y.reshape(BATCH, 1).astype(F32)
    if _trace:
        return out, res
    return out

